# revision 1
# baseline (speedup 1.0000x reference)
"""MoE feed-forward (top-2 routing, E=8 experts) on 8 TRN2 NeuronCores.

Strategy: expert parallelism with host-side routing/dispatch.
  - Host computes the router (softmax + top-2 + renormalize) in float64,
    gathers each expert's tokens (padded to a common capacity C), and
    pre-tiles/pre-transposes all operands into DMA-friendly layouts.
  - Core e runs the GLU MLP for expert e over its C gathered tokens:
      phase 1: hT[H, C] = silu(w1[e] @ xT) * (w3[e] @ xT)   (x resident,
               w1/w3 streamed once, h kept resident in SBUF as bf16)
      phase 2: y[C, D] = (hT)^T @ w2[e]^T, scaled per-token by the
               renormalized routing weight (w2 streamed once)
  - Host scatter-adds the 8 per-expert outputs into the final [T, D].

Matmuls run in bf16 (1 cyc/row on the PE vs 4 for fp32) with fp32 PSUM
accumulation.
"""
import sys

if "/opt/trn_rl_repo" not in sys.path:
    sys.path.insert(0, "/opt/trn_rl_repo")

import numpy as np
import ml_dtypes

import concourse.mybir as mybir
from concourse import bacc
from concourse.tile import TileContext
from concourse.bass_utils import run_bass_kernel_spmd

BF16 = ml_dtypes.bfloat16
P = 128
D = 2048   # model dim
H = 4096   # hidden dim
E = 8      # experts == cores
TOP_K = 2
DO = D // P   # 16 contraction chunks for layer 1
HO = H // P   # 32 contraction chunks for layer 2


def _route(x, router_w):
    """Top-2 expert selection + renormalized weights (float64 host math)."""
    logits = x.astype(np.float64) @ router_w.astype(np.float64).T
    m = logits.max(axis=1, keepdims=True)
    p = np.exp(logits - m)
    p /= p.sum(axis=1, keepdims=True)
    sel = np.argsort(-p, axis=1, kind="stable")[:, :TOP_K]
    rw = np.take_along_axis(p, sel, axis=1)
    rw /= rw.sum(axis=1, keepdims=True)
    return sel, rw.astype(np.float32)


def _build(C):
    """Bass program: one expert's GLU MLP over C gathered tokens."""
    Ct = C // P
    f32 = mybir.dt.float32
    bf16 = mybir.dt.bfloat16

    nc = bacc.Bacc(None, target_bir_lowering=False)
    xthd = nc.dram_tensor("xthd", [P, DO, C], bf16, kind="ExternalInput")
    w1thd = nc.dram_tensor("w1thd", [P, HO, DO, P], bf16, kind="ExternalInput")
    w3thd = nc.dram_tensor("w3thd", [P, HO, DO, P], bf16, kind="ExternalInput")
    w2thd = nc.dram_tensor("w2thd", [P, HO, D], bf16, kind="ExternalInput")
    csv = nc.dram_tensor("csv", [P, Ct], f32, kind="ExternalInput")
    out = nc.dram_tensor("out", [C, D], f32, kind="ExternalOutput")
    out_v = out.rearrange("(tt p) d -> p tt d", p=P)

    tblocks = [(t0, min(512, C - t0)) for t0 in range(0, C, 512)]
    Silu = mybir.ActivationFunctionType.Silu

    with TileContext(nc) as tc:
        with (
            tc.tile_pool(name="resident", bufs=1) as resident,
            tc.tile_pool(name="w13", bufs=3) as w13pool,
            tc.tile_pool(name="silu", bufs=3) as silupool,
            tc.tile_pool(name="w2", bufs=36) as w2pool,
            tc.tile_pool(name="y", bufs=3) as ypool,
            tc.tile_pool(name="ps13", bufs=2, space="PSUM") as ps13,
            tc.tile_pool(name="ps2", bufs=3, space="PSUM") as ps2,
        ):
            xsb = resident.tile([P, DO, C], bf16, name="xsb")
            for do in range(DO):
                nc.sync.dma_start(out=xsb[:, do, :], in_=xthd[:, do, :])
            hsb = resident.tile([P, HO, C], bf16, name="hsb")
            css = resident.tile([P, Ct], f32, name="css")
            nc.sync.dma_start(out=css[:], in_=csv[:])

            # ---- phase 1: hT = silu(w1 xT) * (w3 xT), laid out [h, t] ----
            for ht in range(HO):
                w1t = w13pool.tile([P, DO, P], bf16, name="w1t")
                nc.sync.dma_start(out=w1t[:], in_=w1thd[:, ht, :, :])
                w3t = w13pool.tile([P, DO, P], bf16, name="w3t")
                nc.sync.dma_start(out=w3t[:], in_=w3thd[:, ht, :, :])
                for (t0, tn) in tblocks:
                    pg = ps13.tile([P, 512], f32, name="pg")[:, :tn]
                    pu = ps13.tile([P, 512], f32, name="pu")[:, :tn]
                    for dk in range(DO):
                        nc.tensor.matmul(
                            pg, w1t[:, dk, :], xsb[:, dk, t0 : t0 + tn],
                            start=(dk == 0), stop=(dk == DO - 1),
                        )
                    for dk in range(DO):
                        nc.tensor.matmul(
                            pu, w3t[:, dk, :], xsb[:, dk, t0 : t0 + tn],
                            start=(dk == 0), stop=(dk == DO - 1),
                        )
                    st = silupool.tile([P, 512], f32, name="st")[:, :tn]
                    nc.scalar.activation(st, pg, Silu)
                    nc.vector.tensor_mul(hsb[:, ht, t0 : t0 + tn], st, pu)

            # ---- phase 2: y[t, d] = sum_h hT[h, t] * w2t[h, d], scaled ----
            for dq in range(4):
                dsl = slice(dq * 512, (dq + 1) * 512)
                w2ts = []
                for ho in range(HO):
                    w2t = w2pool.tile([P, 512], bf16, name="w2t")
                    nc.sync.dma_start(out=w2t[:], in_=w2thd[:, ho, dsl])
                    w2ts.append(w2t)
                for tt in range(Ct):
                    py = ps2.tile([P, 512], f32, name="py")
                    for ho in range(HO):
                        nc.tensor.matmul(
                            py, hsb[:, ho, tt * P : (tt + 1) * P], w2ts[ho],
                            start=(ho == 0), stop=(ho == HO - 1),
                        )
                    ysb = ypool.tile([P, 512], f32, name="ysb")
                    nc.vector.tensor_scalar_mul(ysb, py, css[:, tt : tt + 1])
                    nc.sync.dma_start(out=out_v[:, tt, dsl], in_=ysb)

    nc.compile()
    return nc


def _prep_core(x, w1_e, w3_e, w2_e, idx, cw, C):
    """Per-core input arrays in device layouts (see _build docstring)."""
    cnt = len(idx)
    xg = np.zeros((C, D), np.float32)
    xg[:cnt] = x[idx]
    # [p, do, c] with d = do*P + p
    xthd = np.ascontiguousarray(
        xg.T.reshape(DO, P, C).transpose(1, 0, 2).astype(BF16)
    )
    # [p, ht, do, hi] with d = do*P + p, h = ht*P + hi  (from w1 [H, D])
    w1thd = np.ascontiguousarray(
        w1_e.reshape(HO, P, DO, P).transpose(3, 0, 2, 1).astype(BF16)
    )
    w3thd = np.ascontiguousarray(
        w3_e.reshape(HO, P, DO, P).transpose(3, 0, 2, 1).astype(BF16)
    )
    # [p, ho, d] with h = ho*P + p  (from w2 [D, H])
    w2thd = np.ascontiguousarray(
        w2_e.reshape(D, HO, P).transpose(2, 1, 0).astype(BF16)
    )
    csf = np.zeros(C, np.float32)
    csf[:cnt] = cw
    csv = np.ascontiguousarray(csf.reshape(C // P, P).T)
    return {
        "xthd": xthd, "w1thd": w1thd, "w3thd": w3thd,
        "w2thd": w2thd, "csv": csv,
    }


def kernel(x, router_w, w1, w3, w2, _trace=False):
    T = x.shape[0]
    x = np.asarray(x, np.float32)
    router_w = np.asarray(router_w, np.float32)
    w1 = np.asarray(w1, np.float32)
    w3 = np.asarray(w3, np.float32)
    w2 = np.asarray(w2, np.float32)

    sel, rw = _route(x, router_w)
    idxs, cws = [], []
    for e in range(E):
        mask = sel == e  # [T, 2]; a token never selects the same expert twice
        tok = np.nonzero(mask.any(axis=1))[0]
        cw = np.where(mask[tok, 0], rw[tok, 0], rw[tok, 1])
        idxs.append(tok)
        cws.append(cw)

    C = max(len(t) for t in idxs)
    C = max(P, -(-C // P) * P)  # round up to a multiple of 128

    in_maps = [
        _prep_core(x, w1[e], w3[e], w2[e], idxs[e], cws[e], C) for e in range(E)
    ]
    nc = _build(C)
    res = run_bass_kernel_spmd(
        nc, in_maps, core_ids=list(range(E)), trace=_trace
    )

    out = np.zeros((T, D), np.float32)
    for e in range(E):
        out[idxs[e]] += res.results[e]["out"][: len(idxs[e])]
    if _trace:
        kernel.last_exec_time_ns = res.exec_time_ns
        kernel.last_results = res
    return out


# revision 2
# speedup vs baseline: 1.0435x; 1.0435x over previous
"""MoE feed-forward (top-2 routing, E=8 experts) on 8 TRN2 NeuronCores.

Strategy: expert parallelism with host-side routing/dispatch.
  - Host computes the router (softmax + top-2 + renormalize) in float64,
    gathers each expert's tokens (padded to a common capacity), and
    pre-tiles/pre-transposes all operands into DMA-friendly layouts.
  - Core e runs the GLU MLP for expert e over its gathered tokens:
      phase 1: hT[H, Cx] = silu(w1[e] @ xT) * (w3[e] @ xT)   (x resident,
               w1/w3 streamed once, h kept resident in SBUF as bf16)
      phase 2: y[C, D] = (hT)^T @ w2[e]^T, scaled per-token by the
               renormalized routing weight (w2 streamed once)
  - Host scatter-adds the 8 per-expert outputs into the final [T, D].

Matmuls run in bf16 (1 cyc/row on the PE vs 4 for fp32) with fp32 PSUM
accumulation. Phase 1 uses the exact token span Cx (multiple of 8);
phase 2 pads to C (multiple of 128, the PE partition width) — padded
rows are scaled by 0 and discarded on the host.
"""
import sys

if "/opt/trn_rl_repo" not in sys.path:
    sys.path.insert(0, "/opt/trn_rl_repo")

import numpy as np
import ml_dtypes

import concourse.mybir as mybir
from concourse import bacc
from concourse.tile import TileContext
from concourse.bass_utils import run_bass_kernel_spmd

BF16 = ml_dtypes.bfloat16
P = 128
D = 2048   # model dim
H = 4096   # hidden dim
E = 8      # experts == cores
TOP_K = 2
DO = D // P   # 16 contraction chunks for layer 1
HO = H // P   # 32 contraction chunks for layer 2


def _route(x, router_w):
    """Top-2 expert selection + renormalized weights (float64 host math)."""
    logits = x.astype(np.float64) @ router_w.astype(np.float64).T
    m = logits.max(axis=1, keepdims=True)
    p = np.exp(logits - m)
    p /= p.sum(axis=1, keepdims=True)
    sel = np.argsort(-p, axis=1, kind="stable")[:, :TOP_K]
    rw = np.take_along_axis(p, sel, axis=1)
    rw /= rw.sum(axis=1, keepdims=True)
    return sel, rw.astype(np.float32)


def _tblocks(Cx):
    """Split Cx into roughly equal blocks of <=512 columns (all >=256 when
    Cx >= 512, so no matmul is weight-load-bound)."""
    n = -(-Cx // 512)
    base = Cx // n
    sizes = [base + (1 if i < Cx - base * n else 0) for i in range(n)]
    out, t0 = [], 0
    for s in sizes:
        out.append((t0, s))
        t0 += s
    return out


def _build(C, Cx):
    """Bass program: one expert's GLU MLP over its gathered tokens.

    C  -- phase-2 token capacity (multiple of 128)
    Cx -- phase-1 exact token span (multiple of 8, <= C)
    """
    Ct = C // P
    f32 = mybir.dt.float32
    bf16 = mybir.dt.bfloat16

    nc = bacc.Bacc(None, target_bir_lowering=False)
    xthd = nc.dram_tensor("xthd", [P, DO, Cx], bf16, kind="ExternalInput")
    w1thd = nc.dram_tensor("w1thd", [P, HO, DO, P], bf16, kind="ExternalInput")
    w3thd = nc.dram_tensor("w3thd", [P, HO, DO, P], bf16, kind="ExternalInput")
    w2thd = nc.dram_tensor("w2thd", [P, HO, D], bf16, kind="ExternalInput")
    csv = nc.dram_tensor("csv", [P, Ct], f32, kind="ExternalInput")
    out = nc.dram_tensor("out", [C, D], f32, kind="ExternalOutput")
    out_v = out.rearrange("(tt p) d -> p tt d", p=P)

    tblocks = _tblocks(Cx)
    Silu = mybir.ActivationFunctionType.Silu

    with TileContext(nc) as tc:
        with (
            tc.tile_pool(name="resident", bufs=1) as resident,
            tc.tile_pool(name="w13", bufs=3) as w13pool,
            tc.tile_pool(name="silu", bufs=3) as silupool,
            tc.tile_pool(name="w2", bufs=40) as w2pool,
            tc.tile_pool(name="y", bufs=3) as ypool,
            tc.tile_pool(name="ps13", bufs=2, space="PSUM") as ps13,
            tc.tile_pool(name="ps2", bufs=3, space="PSUM") as ps2,
        ):
            xsb = resident.tile([P, DO, Cx], bf16, name="xsb")
            hsb = resident.tile([P, HO, C], bf16, name="hsb")
            css = resident.tile([P, Ct], f32, name="css")

            # Startup-critical DMA order (sync queue is ~FIFO): the first
            # matmuls need w1/w3 of ht=0 and the first t-block of x.
            nc.sync.dma_start(out=css[:], in_=csv[:])
            w13_0 = []
            for name, src in (("w1t", w1thd), ("w3t", w3thd)):
                wt = w13pool.tile([P, DO, P], bf16, name=name)
                nc.sync.dma_start(out=wt[:], in_=src[:, 0, :, :])
                w13_0.append(wt)
            t00, tn0 = tblocks[0]
            for do in range(DO):
                nc.sync.dma_start(
                    out=xsb[:, do, t00 : t00 + tn0], in_=xthd[:, do, t00 : t00 + tn0]
                )
            # Bulk of x on the gpsimd queue so it doesn't block the
            # sync-queue weight stream.
            for (t0, tn) in tblocks[1:]:
                for do in range(DO):
                    nc.gpsimd.dma_start(
                        out=xsb[:, do, t0 : t0 + tn], in_=xthd[:, do, t0 : t0 + tn]
                    )
            # Phase 2 reads hsb in 128-wide t-tiles; zero the pad columns
            # so padded-token garbage can't turn into NaNs (host discards
            # those rows anyway, but keep them finite).
            if C > Cx:
                for ho in range(HO):
                    nc.gpsimd.memset(hsb[:, ho, Cx:C], 0.0)

            # ---- phase 1: hT = silu(w1 xT) * (w3 xT), laid out [h, t] ----
            for ht in range(HO):
                if ht == 0:
                    w1t, w3t = w13_0
                else:
                    w1t = w13pool.tile([P, DO, P], bf16, name="w1t")
                    nc.sync.dma_start(out=w1t[:], in_=w1thd[:, ht, :, :])
                    w3t = w13pool.tile([P, DO, P], bf16, name="w3t")
                    nc.sync.dma_start(out=w3t[:], in_=w3thd[:, ht, :, :])
                for (t0, tn) in tblocks:
                    pg = ps13.tile([P, 512], f32, name="pg")[:, :tn]
                    pu = ps13.tile([P, 512], f32, name="pu")[:, :tn]
                    for dk in range(DO):
                        nc.tensor.matmul(
                            pg, w1t[:, dk, :], xsb[:, dk, t0 : t0 + tn],
                            start=(dk == 0), stop=(dk == DO - 1),
                        )
                    for dk in range(DO):
                        nc.tensor.matmul(
                            pu, w3t[:, dk, :], xsb[:, dk, t0 : t0 + tn],
                            start=(dk == 0), stop=(dk == DO - 1),
                        )
                    st = silupool.tile([P, 512], f32, name="st")[:, :tn]
                    nc.scalar.activation(st, pg, Silu)
                    nc.vector.tensor_mul(hsb[:, ht, t0 : t0 + tn], st, pu)

            # ---- phase 2: y[t, d] = sum_h hT[h, t] * w2t[h, d], scaled ----
            for dq in range(4):
                dsl = slice(dq * 512, (dq + 1) * 512)
                w2ts = []
                for ho in range(HO):
                    w2t = w2pool.tile([P, 512], bf16, name="w2t")
                    nc.sync.dma_start(out=w2t[:], in_=w2thd[:, ho, dsl])
                    w2ts.append(w2t)
                for tt in range(Ct):
                    py = ps2.tile([P, 512], f32, name="py")
                    for ho in range(HO):
                        nc.tensor.matmul(
                            py, hsb[:, ho, tt * P : (tt + 1) * P], w2ts[ho],
                            start=(ho == 0), stop=(ho == HO - 1),
                        )
                    ysb = ypool.tile([P, 512], f32, name="ysb")
                    nc.vector.tensor_scalar_mul(ysb, py, css[:, tt : tt + 1])
                    nc.sync.dma_start(out=out_v[:, tt, dsl], in_=ysb)

    nc.compile()
    return nc


def _prep_core(x, w1_e, w3_e, w2_e, idx, cw, C, Cx):
    """Per-core input arrays in device layouts (see _build docstring)."""
    cnt = len(idx)
    xg = np.zeros((Cx, D), np.float32)
    xg[:cnt] = x[idx]
    # [p, do, c] with d = do*P + p
    xthd = np.ascontiguousarray(
        xg.T.reshape(DO, P, Cx).transpose(1, 0, 2).astype(BF16)
    )
    # [p, ht, do, hi] with d = do*P + p, h = ht*P + hi  (from w1 [H, D])
    w1thd = np.ascontiguousarray(
        w1_e.reshape(HO, P, DO, P).transpose(3, 0, 2, 1).astype(BF16)
    )
    w3thd = np.ascontiguousarray(
        w3_e.reshape(HO, P, DO, P).transpose(3, 0, 2, 1).astype(BF16)
    )
    # [p, ho, d] with h = ho*P + p  (from w2 [D, H])
    w2thd = np.ascontiguousarray(
        w2_e.reshape(D, HO, P).transpose(2, 1, 0).astype(BF16)
    )
    csf = np.zeros(C, np.float32)
    csf[:cnt] = cw
    csv = np.ascontiguousarray(csf.reshape(C // P, P).T)
    return {
        "xthd": xthd, "w1thd": w1thd, "w3thd": w3thd,
        "w2thd": w2thd, "csv": csv,
    }


def kernel(x, router_w, w1, w3, w2, _trace=False):
    T = x.shape[0]
    x = np.asarray(x, np.float32)
    router_w = np.asarray(router_w, np.float32)
    w1 = np.asarray(w1, np.float32)
    w3 = np.asarray(w3, np.float32)
    w2 = np.asarray(w2, np.float32)

    sel, rw = _route(x, router_w)
    idxs, cws = [], []
    for e in range(E):
        mask = sel == e  # [T, 2]; a token never selects the same expert twice
        tok = np.nonzero(mask.any(axis=1))[0]
        cw = np.where(mask[tok, 0], rw[tok, 0], rw[tok, 1])
        idxs.append(tok)
        cws.append(cw)

    mx = max(len(t) for t in idxs)
    Cx = max(8, -(-mx // 8) * 8)     # exact span, multiple of 8
    C = max(P, -(-mx // P) * P)      # phase-2 capacity, multiple of 128

    in_maps = [
        _prep_core(x, w1[e], w3[e], w2[e], idxs[e], cws[e], C, Cx)
        for e in range(E)
    ]
    nc = _build(C, Cx)
    res = run_bass_kernel_spmd(
        nc, in_maps, core_ids=list(range(E)), trace=_trace
    )

    out = np.zeros((T, D), np.float32)
    for e in range(E):
        out[idxs[e]] += res.results[e]["out"][: len(idxs[e])]
    if _trace:
        kernel.last_exec_time_ns = res.exec_time_ns
        kernel.last_results = res
    return out
